# revision 17
# baseline (speedup 1.0000x reference)
"""Trainium2 Bass kernel for nn_Backward_14328010900205.

Flow-warp (grid_sample bilinear, zero padding, align_corners=True) with a
hard validity mask, matching the jax reference:

    (y, x) = (i + fy[b,i,j], j + fx[b,i,j])
    out[b,c,i,j] = mask(b,i,j) * sum_k w_k(b,i,j) * img[b,c, y_k, x_k]

Sharding: pure data parallel, one batch sample per NeuronCore (B=8 -> 8 cores).

Device algorithm (per core):
  - Host ships a pixel-major fp16 copy of the sample: imgT[y*W+x, c],
    channels padded 196 -> 256 so each pixel row is 512B.
  - Map phase (DVE, natural [row, col] layout): bilinear corner weights with
    zero-pad masking folded in, and flat gather indices idx = ys*256+xs plus
    corner offsets {0, 1, 256, 257}.
  - PE transposes rearrange the natural-layout maps into (a) the 16-partition
    "wrapped" int16 index layout dma_gather wants and (b) per-block weight
    columns WT[:, blk].
  - dma_gather (SWDGE descriptor gen on GPSIMD, transfer on the 16 DMA
    engines) fetches one 512B pixel row per (pixel, corner) from HBM:
    partition k of gather block blk holds corner k%4 of pixel k//4.
  - Weighting: G *= WT[:, blk] broadcast along free dim (alternating DVE /
    Scalar engine), then one matmul per block with a constant one-hot
    S[k, m] = [k//4 == m] reduces the 4 corners: psum[m, ch].
  - PSUM -> DRAM DMA assembles the pixel-major f32 output outT[px, c];
    host transposes back to [C, H, W].
"""

import numpy as np

from concourse import bacc, bass, mybir, tile

AOT = mybir.AluOpType
F32 = mybir.dt.float32
F16 = mybir.dt.float16
I16 = mybir.dt.int16

B, C, H, W = 8, 196, 128, 256
HW = H * W
CP = 256          # padded channel count (512B fp16 rows)
N_CORES = 8

# gather/block geometry
NU = 8            # transpose blocks: u indexes groups of 32 pixels per row
NRQ = 4           # row-quarters per u  -> 32 gather chunks per sample
ROWS_PER_CHUNK = H // NRQ          # 32 blocks (rows) per chunk
IDX_PER_ROW = 128                  # 32 px * 4 corners
CHUNK_IDX = ROWS_PER_CHUNK * IDX_PER_ROW   # 4096 indices per dma_gather
SCOLS = HW * 2 // 16               # 4096 wrapped idx columns (2 idx/px)


def split_drain_waits(nc, max_waits=1):
    """walrus CoreV3 codegen rejects instructions carrying more than a
    couple of sync waits; hoist extras onto preceding NoOps."""
    fn = nc.main_func
    n = 0
    for bb in fn.blocks:
        insts = bb.instructions
        i = 0
        while i < len(insts):
            ins = insts[i]
            if type(ins).__name__ != "InstNoOp":
                si = ins.sync_info
                ow = list(si.on_wait) if (si and si.on_wait) else []
                if len(ow) > max_waits:
                    keep = ow[-max_waits:]
                    extras = ow[:-max_waits]
                    ins.sync_info = si.__replace__(on_wait=keep)
                    for k, wt in enumerate(extras):
                        nop = mybir.InstNoOp(
                            name=f"{ins.name}-wsplit{k}",
                            engine=ins.engine,
                            ins=[],
                            outs=[],
                            sync_info=mybir.SyncInfo(on_wait=[wt], on_update=[]),
                        )
                        insts.insert(i, nop)
                        i += 1
                        n += 1
            i += 1
    return n


class Slots:
    """Column-sliced scratch slots inside one SBUF tile."""

    def __init__(self, tilebuf, width):
        self.t = tilebuf
        self.w = width

    def __getitem__(self, k):
        return self.t[:, k * self.w:(k + 1) * self.w]


def build_program(n_iters=1, split_drains=True, skip=()):
    nc = bacc.Bacc("TRN2", target_bir_lowering=False, debug=False,
                   num_swdge_queues=4)

    imgT = nc.declare_dram_parameter("imgT", [HW, CP], F16, isOutput=False)
    flow = nc.declare_dram_parameter("flow", [2, H, W], F32, isOutput=False)
    Jc = nc.declare_dram_parameter("cJ", [128, 256], F32, isOutput=False)
    Ic = nc.declare_dram_parameter("cI", [128, 256], F32, isOutput=False)
    IDc = nc.declare_dram_parameter("cID", [128, 128], F32, isOutput=False)
    Sc = nc.declare_dram_parameter("cS", [128, 64], F16, isOutput=False)
    outT = nc.declare_dram_parameter("outT", [HW, C], F32, isOutput=True)

    with tile.TileContext(nc) as tc:
        with (
            tc.tile_pool(name="consts", bufs=1) as cpool,
            tc.tile_pool(name="maps", bufs=1) as mpool,
            tc.tile_pool(name="gath", bufs=3) as gpool,
            tc.tile_pool(name="outp", bufs=4) as opool,
            tc.tile_pool(name="pst", bufs=2, space="PSUM") as tpool,
            tc.tile_pool(name="psb", bufs=3, space="PSUM") as bpool,
        ):
            cs = {}
            for name, dram, shape, dt in (
                ("J", Jc, [128, 256], F32),
                ("I", Ic, [128, 256], F32),
                ("ID", IDc, [128, 128], F32),
                ("S", Sc, [128, 64], F16),
            ):
                cs[name] = cpool.tile(shape, dt, tag="c" + name, name="c" + name)
                nc.sync.dma_start(out=cs[name][:, :], in_=dram[:, :])

            for _ in range(n_iters):
                _iteration(nc, mpool, gpool, opool, tpool, bpool,
                           imgT, flow, outT, cs, skip)

    nc.compile()  # lowers pseudo-insts, inserts GPSIMD library loads
    if split_drains:
        split_drain_waits(nc)
    return nc


def _iteration(nc, mpool, gpool, opool, tpool, bpool, imgT, flow, outT, cs,
               skip=()):
    ts = nc.vector.tensor_scalar
    tt = nc.vector.tensor_tensor

    # ---------------- map phase (natural [row, col] layout) ----------------
    mb = Slots(mpool.tile([128, 14 * 256], F32, tag="mapbuf", name="mapbuf"), 256)
    Mw0 = mpool.tile([128, 512], F32, tag="Mw0", name="Mw0")
    Mw1 = mpool.tile([128, 512], F32, tag="Mw1", name="Mw1")
    Mi = mpool.tile([128, 512], F32, tag="Mi", name="Mi")
    WT0 = mpool.tile([128, 512], F16, tag="WT0", name="WT0")
    WT1 = mpool.tile([128, 512], F16, tag="WT1", name="WT1")
    IX = mpool.tile([128, SCOLS], I16, tag="IX", name="IX")

    X, Y, AX, AY, X0, Y0 = mb[0], mb[1], mb[2], mb[3], mb[4], mb[5]
    P0X, P1X, P0Y, P1Y, MSK = mb[6], mb[7], mb[8], mb[9], mb[10]
    S1, S2, S3 = mb[11], mb[12], mb[13]
    WSA, WSB = P0X, P1X  # reused after pair weights are folded

    nc.sync.dma_start(out=X, in_=flow[0])
    nc.sync.dma_start(out=Y, in_=flow[1])

    tt(out=X, in0=X, in1=cs["J"][:, :], op=AOT.add)          # x = j + fx
    tt(out=Y, in0=Y, in1=cs["I"][:, :], op=AOT.add)          # y = i + fy
    # floor via round-to-nearest bias trick: r = (x + 2^23) - 2^23;
    # floor = r - [r > x]; frac = x - floor
    ts(X0, X, 8388608.0, 8388608.0, AOT.add, AOT.subtract)   # round(x)
    tt(out=AX, in0=X0, in1=X, op=AOT.is_gt)
    tt(out=X0, in0=X0, in1=AX, op=AOT.subtract)              # floor(x)
    tt(out=AX, in0=X, in1=X0, op=AOT.subtract)               # frac(x)
    ts(Y0, Y, 8388608.0, 8388608.0, AOT.add, AOT.subtract)
    tt(out=AY, in0=Y0, in1=Y, op=AOT.is_gt)
    tt(out=Y0, in0=Y0, in1=AY, op=AOT.subtract)
    tt(out=AY, in0=Y, in1=Y0, op=AOT.subtract)

    # p0x = (1-ax)*[0<=x0<=255], p1x = ax*[-1<=x0<=254]
    ts(S1, X0, 0.0, 255.0, AOT.max, AOT.min)
    tt(out=S1, in0=X0, in1=S1, op=AOT.is_equal)
    ts(S2, AX, -1.0, 1.0, AOT.mult, AOT.add)
    tt(out=P0X, in0=S1, in1=S2, op=AOT.mult)
    ts(S1, X0, -1.0, 254.0, AOT.max, AOT.min)
    tt(out=S1, in0=X0, in1=S1, op=AOT.is_equal)
    tt(out=P1X, in0=AX, in1=S1, op=AOT.mult)
    # p0y, p1y
    ts(S1, Y0, 0.0, 127.0, AOT.max, AOT.min)
    tt(out=S1, in0=Y0, in1=S1, op=AOT.is_equal)
    ts(S2, AY, -1.0, 1.0, AOT.mult, AOT.add)
    tt(out=P0Y, in0=S1, in1=S2, op=AOT.mult)
    ts(S1, Y0, -1.0, 126.0, AOT.max, AOT.min)
    tt(out=S1, in0=Y0, in1=S1, op=AOT.is_equal)
    tt(out=P1Y, in0=AY, in1=S1, op=AOT.mult)

    # mask = [(p0x+p1x)*(p0y+p1y) > 0.999]
    tt(out=S1, in0=P0X, in1=P1X, op=AOT.add)
    tt(out=S2, in0=P0Y, in1=P1Y, op=AOT.add)
    tt(out=S1, in0=S1, in1=S2, op=AOT.mult)
    ts(MSK, S1, 0.999, None, AOT.is_gt)

    # slot-x weights at xs = clip(x0, 0, 254):
    #   wsx0 = p0x*[x0==xs] + p1x*[x0==-1]
    #   wsx1 = p1x*[x0==xs] + p0x*[x0==255]
    ts(S1, X0, 0.0, 254.0, AOT.max, AOT.min)                 # xs (kept in S1? no)
    XS = AX  # frac no longer needed; reuse as xs
    ts(XS, X0, 0.0, 254.0, AOT.max, AOT.min)                 # xs
    tt(out=S1, in0=X0, in1=XS, op=AOT.is_equal)              # e0x
    tt(out=S2, in0=P0X, in1=S1, op=AOT.mult)
    ts(S3, X0, -1.0, None, AOT.is_equal)
    tt(out=S3, in0=P1X, in1=S3, op=AOT.mult)
    tt(out=S2, in0=S2, in1=S3, op=AOT.add)                   # wsx0 raw
    tt(out=S1, in0=P1X, in1=S1, op=AOT.mult)                 # p1x*e0x
    ts(S3, X0, 255.0, None, AOT.is_equal)
    tt(out=S3, in0=P0X, in1=S3, op=AOT.mult)
    tt(out=S1, in0=S1, in1=S3, op=AOT.add)                   # wsx1 raw
    nc.vector.tensor_copy(out=WSA, in_=S2)                   # overwrites P0X
    nc.vector.tensor_copy(out=WSB, in_=S1)                   # overwrites P1X

    # slot-y weights (mask folded) at ys = clip(y0, 0, 126):
    #   wy0 = (p0y*[y0==ys] + p1y*[y0==-1]) * m
    #   wy1 = (p1y*[y0==ys] + p0y*[y0==127]) * m
    YS = AY
    ts(YS, Y0, 0.0, 126.0, AOT.max, AOT.min)                 # ys
    tt(out=S1, in0=Y0, in1=YS, op=AOT.is_equal)              # e0y
    tt(out=S2, in0=P0Y, in1=S1, op=AOT.mult)
    ts(S3, Y0, -1.0, None, AOT.is_equal)
    tt(out=S3, in0=P1Y, in1=S3, op=AOT.mult)
    tt(out=S2, in0=S2, in1=S3, op=AOT.add)                   # wy0 raw
    tt(out=S1, in0=P1Y, in1=S1, op=AOT.mult)                 # p1y*e0y
    ts(S3, Y0, 127.0, None, AOT.is_equal)
    tt(out=S3, in0=P0Y, in1=S3, op=AOT.mult)
    tt(out=S1, in0=S1, in1=S3, op=AOT.add)                   # wy1 raw
    tt(out=P0Y, in0=S2, in1=MSK, op=AOT.mult)                # wy0
    tt(out=P1Y, in0=S1, in1=MSK, op=AOT.mult)                # wy1
    WY0, WY1 = P0Y, P1Y

    # base flat index = ys*256 + xs
    BASE = S2
    ts(BASE, YS, 256.0, None, AOT.mult)
    tt(out=BASE, in0=BASE, in1=XS, op=AOT.add)

    # pair planes, col = 2*j + h (h = y-corner); each gathered record is
    # the (x0, x0+1) pixel-row pair, so x-slot weights live in two planes
    W0v = Mw0[:, :].rearrange("p (j h) -> p j h", h=2)
    tt(out=W0v[:, :, 0], in0=WY0, in1=WSA, op=AOT.mult)
    tt(out=W0v[:, :, 1], in0=WY1, in1=WSA, op=AOT.mult)
    W1v = Mw1[:, :].rearrange("p (j h) -> p j h", h=2)
    tt(out=W1v[:, :, 0], in0=WY0, in1=WSB, op=AOT.mult)
    tt(out=W1v[:, :, 1], in0=WY1, in1=WSB, op=AOT.mult)
    Iv = Mi[:, :].rearrange("p (j h) -> p j h", h=2)
    ts(Iv[:, :, 0], BASE, 0.0, None, AOT.add)
    ts(Iv[:, :, 1], BASE, 256.0, None, AOT.add)

    # ------------- transpose maps into gather layouts -------------
    # weights: WT{0,1}[:, 128u + r] = w at partition delta = 2*(j-64u)+h
    for u in range(4):
        for Mwp, WTp in ((Mw0, WT0), (Mw1, WT1)):
            psw = tpool.tile([128, 128], F32, tag="psw", name="psw")
            nc.tensor.transpose(psw[:, :], Mwp[:, 128 * u:128 * (u + 1)],
                                cs["ID"][:, :])
            nc.vector.tensor_copy(out=WTp[:, 128 * u:128 * (u + 1)],
                                  in_=psw[:, :])
        # indices: for each 16-col group g: psum[p16, r] -> strided int16
        # columns s = 8r + g of the wrapped layout (per-u base 1024u)
        for g in range(8):
            psi = tpool.tile([16, 128], F32, tag="psi", name="psi")
            c0 = 128 * u + 16 * g
            nc.tensor.transpose(psi[:, :], Mi[:, c0:c0 + 16], cs["ID"][:, :])
            dst = IX[0:16, 1024 * u:1024 * (u + 1)].rearrange(
                "p (r g) -> p r g", g=8)
            nc.vector.tensor_copy(out=dst[:, :, g], in_=psi[:, :])
    # replicate the 16-partition wrapped indices to all 8 core groups
    for k in range(1, 8):
        nc.sync.dma_start(out=IX[16 * k:16 * (k + 1), :], in_=IX[0:16, :])

    # ---------------- gather + blend ----------------
    if "gather" in skip:
        return
    # SWDGE caps one gather at ~1024 descriptors (2048 wedges the ring) ->
    # chunk = 1024 indices = 8 blocks (output rows); each gathered record is
    # the 1KB (x0, x0+1) pixel-row pair via an overlapping-stride source AP.
    inA = imgT[:, :]
    in_pair = bass.AP(inA.tensor, inA.offset, [[256, HW - 1], [1, 512]])
    for u in range(4):
        # x-slot weights folded into the stationary one-hots:
        # SW{0,1}[p, 64*blk + m] = S[p, m] * WT{0,1}[p, 128u + blk]
        SW0 = gpool.tile([128, 8192], F16, tag="SW0", name="SW0")
        SW1 = gpool.tile([128, 8192], F16, tag="SW1", name="SW1")
        if "wmul" not in skip:
            for SWp, WTp in ((SW0, WT0), (SW1, WT1)):
                sv = cs["S"][:, :].rearrange("p (a m) -> p a m", a=1)
                wv = WTp[:, 128 * u:128 * (u + 1)].rearrange(
                    "p (b o) -> p b o", o=1)
                b0, b1 = bass.broadcast_tensor_aps(sv, wv)
                nc.vector.tensor_tensor(
                    out=SWp[:, :].rearrange("p (b m) -> p b m", m=64),
                    in0=b0, in1=b1, op=AOT.mult)
        else:
            nc.vector.memset(SW0[:, 0:64], 0.0)
            nc.vector.memset(SW1[:, 0:64], 0.0)
        for R in range(16):
            G = gpool.tile([128, 8, 512], F16, tag="G", name="G")
            scol = 1024 * u + 64 * R
            if "dg" in skip:
                nc.vector.memset(G[:, 0, 0:4], 0.0)  # keep tile alive
            else:
                nc.gpsimd.dma_gather(
                    out_ap=G[:, :, :],
                    in_ap=in_pair,
                    idxs_ap=IX[:, scol:scol + 64],
                    num_idxs=1024,
                    num_idxs_reg=1024,
                    elem_size=512,
                    elem_step=256,
                    queue_num=(16 * u + R) % 4,
                )
            if "blend" in skip:
                continue
            for half in range(2):
                psb = bpool.tile([128, 512], F32, tag="psb", name="psb")
                for q4 in range(4):
                    q = 4 * half + q4
                    blk = 8 * R + q
                    out_ap = psb[64 * (q4 % 2):64 * (q4 % 2) + 64,
                                 C * (q4 // 2):C * (q4 // 2) + C]
                    nc.tensor.matmul(
                        out=out_ap,
                        lhsT=SW0[:, 64 * blk:64 * blk + 64],
                        rhs=G[:, q, 0:C],
                        start=True, stop=False,
                        tile_position=(0, 64 * (q4 % 2)))
                    nc.tensor.matmul(
                        out=out_ap,
                        lhsT=SW1[:, 64 * blk:64 * blk + 64],
                        rhs=G[:, q, 256:256 + C],
                        start=False, stop=True,
                        tile_position=(0, 64 * (q4 % 2)))
                # drain PSUM through SBUF; block q4 = 2*qc + qp sits at
                # partitions 64*qp, col group qc, covering outT rows
                # 512*a + 256*qp + 64*u + [0, 64) with a = 4R + 2*half + qc
                ob = opool.tile([128, 2 * C], F32, tag="OB", name="OB")
                if R % 2 == 0:
                    nc.scalar.copy(out=ob[:, :], in_=psb[:, 0:2 * C])
                else:
                    nc.vector.tensor_copy(out=ob[:, :], in_=psb[:, 0:2 * C])
                for qc in range(2):
                    a = 4 * R + 2 * half + qc
                    dstv = outT[:, :].rearrange(
                        "(a qp u m) c -> a u qp m c", qp=2, u=4, m=64)
                    nc.sync.dma_start(out=dstv[a, u],
                                      in_=ob[:, C * qc:C * qc + C])


# ---------------- host side ----------------

_CONSTS = None
_PROGRAM = None


def _host_consts():
    global _CONSTS
    if _CONSTS is None:
        j = np.broadcast_to(np.arange(W, dtype=np.float32), (128, W)).copy()
        i = np.broadcast_to(np.arange(H, dtype=np.float32)[:, None],
                            (H, W)).copy()
        ident = np.eye(128, dtype=np.float32)
        s = np.zeros((128, 64), np.float16)
        s[np.arange(128), np.arange(128) // 2] = 1.0
        _CONSTS = {"cJ": j, "cI": i, "cID": ident, "cS": s}
    return _CONSTS


def make_in_maps(tensorInput, tensorFlow):
    consts = _host_consts()
    in_maps = []
    for b in range(B):
        imgT = np.zeros((HW, CP), np.float16)
        imgT[:, :C] = tensorInput[b].reshape(C, HW).T
        m = {"imgT": imgT,
             "flow": np.ascontiguousarray(tensorFlow[b])}
        m.update(consts)
        in_maps.append(m)
    return in_maps


def kernel(tensorInput, tensorFlow):
    from concourse.bass_utils import run_bass_kernel_spmd

    tensorInput = np.asarray(tensorInput, dtype=np.float32)
    tensorFlow = np.asarray(tensorFlow, dtype=np.float32)
    assert tensorInput.shape == (B, C, H, W)
    assert tensorFlow.shape == (B, 2, H, W)

    global _PROGRAM
    if _PROGRAM is None:
        _PROGRAM = build_program(n_iters=1)
    nc = _PROGRAM

    res = run_bass_kernel_spmd(nc, make_in_maps(tensorInput, tensorFlow),
                               list(range(N_CORES)))
    return np.stack([np.ascontiguousarray(
        res.results[b]["outT"].T).reshape(C, H, W) for b in range(B)], axis=0)


# revision 19
# speedup vs baseline: 1.2991x; 1.2991x over previous
"""Trainium2 Bass kernel for nn_Backward_14328010900205.

Flow-warp (grid_sample bilinear, zero padding, align_corners=True) with a
hard validity mask, matching the jax reference:

    (y, x) = (i + fy[b,i,j], j + fx[b,i,j])
    out[b,c,i,j] = mask(b,i,j) * sum_k w_k(b,i,j) * img[b,c, y_k, x_k]

Sharding: pure data parallel, one batch sample per NeuronCore (B=8 -> 8 cores).

Device algorithm (per core):
  - Host ships a pixel-major fp16 copy of the sample: imgT[y*W+x, c],
    channels padded 196 -> 256 so each pixel row is 512B.
  - Map phase (DVE, natural [row, col] layout): bilinear corner weights with
    zero-pad masking folded in, and flat gather indices idx = ys*256+xs plus
    corner offsets {0, 1, 256, 257}.
  - PE transposes rearrange the natural-layout maps into (a) the 16-partition
    "wrapped" int16 index layout dma_gather wants and (b) per-block weight
    columns WT[:, blk].
  - dma_gather (SWDGE descriptor gen on GPSIMD, transfer on the 16 DMA
    engines) fetches one 512B pixel row per (pixel, corner) from HBM:
    partition k of gather block blk holds corner k%4 of pixel k//4.
  - Weighting: G *= WT[:, blk] broadcast along free dim (alternating DVE /
    Scalar engine), then one matmul per block with a constant one-hot
    S[k, m] = [k//4 == m] reduces the 4 corners: psum[m, ch].
  - PSUM -> DRAM DMA assembles the pixel-major f32 output outT[px, c];
    host transposes back to [C, H, W].
"""

import numpy as np

from concourse import bacc, bass, mybir, tile

AOT = mybir.AluOpType
F32 = mybir.dt.float32
F16 = mybir.dt.float16
I16 = mybir.dt.int16

B, C, H, W = 8, 196, 128, 256
HW = H * W
CP = 256          # padded channel count (512B fp16 rows)
N_CORES = 8

# gather/block geometry
NU = 8            # transpose blocks: u indexes groups of 32 pixels per row
NRQ = 4           # row-quarters per u  -> 32 gather chunks per sample
ROWS_PER_CHUNK = H // NRQ          # 32 blocks (rows) per chunk
IDX_PER_ROW = 128                  # 32 px * 4 corners
CHUNK_IDX = ROWS_PER_CHUNK * IDX_PER_ROW   # 4096 indices per dma_gather
SCOLS = HW * 4 // 16               # 8192 wrapped idx columns per sample


def split_drain_waits(nc, max_waits=1):
    """walrus CoreV3 codegen rejects instructions carrying more than a
    couple of sync waits; hoist extras onto preceding NoOps."""
    fn = nc.main_func
    n = 0
    for bb in fn.blocks:
        insts = bb.instructions
        i = 0
        while i < len(insts):
            ins = insts[i]
            if type(ins).__name__ != "InstNoOp":
                si = ins.sync_info
                ow = list(si.on_wait) if (si and si.on_wait) else []
                if len(ow) > max_waits:
                    keep = ow[-max_waits:]
                    extras = ow[:-max_waits]
                    ins.sync_info = si.__replace__(on_wait=keep)
                    for k, wt in enumerate(extras):
                        nop = mybir.InstNoOp(
                            name=f"{ins.name}-wsplit{k}",
                            engine=ins.engine,
                            ins=[],
                            outs=[],
                            sync_info=mybir.SyncInfo(on_wait=[wt], on_update=[]),
                        )
                        insts.insert(i, nop)
                        i += 1
                        n += 1
            i += 1
    return n


class Slots:
    """Column-sliced scratch slots inside one SBUF tile."""

    def __init__(self, tilebuf, width):
        self.t = tilebuf
        self.w = width

    def __getitem__(self, k):
        return self.t[:, k * self.w:(k + 1) * self.w]


def build_program(n_iters=1, split_drains=True, skip=()):
    nc = bacc.Bacc("TRN2", target_bir_lowering=False, debug=False,
                   num_swdge_queues=4)

    imgT = nc.declare_dram_parameter("imgT", [HW, CP], F16, isOutput=False)
    flow = nc.declare_dram_parameter("flow", [2, H, W], F32, isOutput=False)
    Jc = nc.declare_dram_parameter("cJ", [128, 256], F32, isOutput=False)
    Ic = nc.declare_dram_parameter("cI", [128, 256], F32, isOutput=False)
    IDc = nc.declare_dram_parameter("cID", [128, 128], F32, isOutput=False)
    Sc = nc.declare_dram_parameter("cS", [128, 32], F16, isOutput=False)
    outT = nc.declare_dram_parameter("outT", [HW, C], F32, isOutput=True)

    with tile.TileContext(nc) as tc:
        with (
            tc.tile_pool(name="consts", bufs=1) as cpool,
            tc.tile_pool(name="maps", bufs=1) as mpool,
            tc.tile_pool(name="gath", bufs=6) as gpool,
            tc.tile_pool(name="outp", bufs=6) as opool,
            tc.tile_pool(name="pst", bufs=2, space="PSUM") as tpool,
            tc.tile_pool(name="psb", bufs=4, space="PSUM") as bpool,
        ):
            cs = {}
            for name, dram, shape, dt in (
                ("J", Jc, [128, 256], F32),
                ("I", Ic, [128, 256], F32),
                ("ID", IDc, [128, 128], F32),
                ("S", Sc, [128, 32], F16),
            ):
                cs[name] = cpool.tile(shape, dt, tag="c" + name, name="c" + name)
                nc.sync.dma_start(out=cs[name][:, :], in_=dram[:, :])

            for _ in range(n_iters):
                _iteration(nc, mpool, gpool, opool, tpool, bpool,
                           imgT, flow, outT, cs, skip)

    nc.compile()  # lowers pseudo-insts, inserts GPSIMD library loads
    if split_drains:
        split_drain_waits(nc)
    return nc


def _iteration(nc, mpool, gpool, opool, tpool, bpool, imgT, flow, outT, cs,
               skip=()):
    ts = nc.vector.tensor_scalar
    tt = nc.vector.tensor_tensor

    # ---------------- map phase (natural [row, col] layout) ----------------
    mb = Slots(mpool.tile([128, 14 * 256], F32, tag="mapbuf", name="mapbuf"), 256)
    Mw = mpool.tile([128, 1024], F32, tag="Mw", name="Mw")
    Mi = mpool.tile([128, 1024], F32, tag="Mi", name="Mi")
    WT = mpool.tile([128, 1024], F16, tag="WT", name="WT")
    IX = mpool.tile([128, SCOLS], I16, tag="IX", name="IX")

    X, Y, AX, AY, X0, Y0 = mb[0], mb[1], mb[2], mb[3], mb[4], mb[5]
    P0X, P1X, P0Y, P1Y, MSK = mb[6], mb[7], mb[8], mb[9], mb[10]
    S1, S2, S3 = mb[11], mb[12], mb[13]
    WSA, WSB = P0X, P1X  # reused after pair weights are folded

    nc.sync.dma_start(out=X, in_=flow[0])
    nc.sync.dma_start(out=Y, in_=flow[1])

    tt(out=X, in0=X, in1=cs["J"][:, :], op=AOT.add)          # x = j + fx
    tt(out=Y, in0=Y, in1=cs["I"][:, :], op=AOT.add)          # y = i + fy
    # floor via round-to-nearest bias trick: r = (x + 2^23) - 2^23;
    # floor = r - [r > x]; frac = x - floor
    ts(X0, X, 8388608.0, 8388608.0, AOT.add, AOT.subtract)   # round(x)
    tt(out=AX, in0=X0, in1=X, op=AOT.is_gt)
    tt(out=X0, in0=X0, in1=AX, op=AOT.subtract)              # floor(x)
    tt(out=AX, in0=X, in1=X0, op=AOT.subtract)               # frac(x)
    ts(Y0, Y, 8388608.0, 8388608.0, AOT.add, AOT.subtract)
    tt(out=AY, in0=Y0, in1=Y, op=AOT.is_gt)
    tt(out=Y0, in0=Y0, in1=AY, op=AOT.subtract)
    tt(out=AY, in0=Y, in1=Y0, op=AOT.subtract)

    # p0x = (1-ax)*[0<=x0<=255], p1x = ax*[-1<=x0<=254]
    ts(S1, X0, 0.0, 255.0, AOT.max, AOT.min)
    tt(out=S1, in0=X0, in1=S1, op=AOT.is_equal)
    ts(S2, AX, -1.0, 1.0, AOT.mult, AOT.add)
    tt(out=P0X, in0=S1, in1=S2, op=AOT.mult)
    ts(S1, X0, -1.0, 254.0, AOT.max, AOT.min)
    tt(out=S1, in0=X0, in1=S1, op=AOT.is_equal)
    tt(out=P1X, in0=AX, in1=S1, op=AOT.mult)
    # p0y, p1y
    ts(S1, Y0, 0.0, 127.0, AOT.max, AOT.min)
    tt(out=S1, in0=Y0, in1=S1, op=AOT.is_equal)
    ts(S2, AY, -1.0, 1.0, AOT.mult, AOT.add)
    tt(out=P0Y, in0=S1, in1=S2, op=AOT.mult)
    ts(S1, Y0, -1.0, 126.0, AOT.max, AOT.min)
    tt(out=S1, in0=Y0, in1=S1, op=AOT.is_equal)
    tt(out=P1Y, in0=AY, in1=S1, op=AOT.mult)

    # mask = [(p0x+p1x)*(p0y+p1y) > 0.999]
    tt(out=S1, in0=P0X, in1=P1X, op=AOT.add)
    tt(out=S2, in0=P0Y, in1=P1Y, op=AOT.add)
    tt(out=S1, in0=S1, in1=S2, op=AOT.mult)
    ts(MSK, S1, 0.999, None, AOT.is_gt)

    # slot-x weights at xs = clip(x0, 0, 254):
    #   wsx0 = p0x*[x0==xs] + p1x*[x0==-1]
    #   wsx1 = p1x*[x0==xs] + p0x*[x0==255]
    ts(S1, X0, 0.0, 254.0, AOT.max, AOT.min)                 # xs (kept in S1? no)
    XS = AX  # frac no longer needed; reuse as xs
    ts(XS, X0, 0.0, 254.0, AOT.max, AOT.min)                 # xs
    tt(out=S1, in0=X0, in1=XS, op=AOT.is_equal)              # e0x
    tt(out=S2, in0=P0X, in1=S1, op=AOT.mult)
    ts(S3, X0, -1.0, None, AOT.is_equal)
    tt(out=S3, in0=P1X, in1=S3, op=AOT.mult)
    tt(out=S2, in0=S2, in1=S3, op=AOT.add)                   # wsx0 raw
    tt(out=S1, in0=P1X, in1=S1, op=AOT.mult)                 # p1x*e0x
    ts(S3, X0, 255.0, None, AOT.is_equal)
    tt(out=S3, in0=P0X, in1=S3, op=AOT.mult)
    tt(out=S1, in0=S1, in1=S3, op=AOT.add)                   # wsx1 raw
    nc.vector.tensor_copy(out=WSA, in_=S2)                   # overwrites P0X
    nc.vector.tensor_copy(out=WSB, in_=S1)                   # overwrites P1X

    # slot-y weights (mask folded) at ys = clip(y0, 0, 126):
    #   wy0 = (p0y*[y0==ys] + p1y*[y0==-1]) * m
    #   wy1 = (p1y*[y0==ys] + p0y*[y0==127]) * m
    YS = AY
    ts(YS, Y0, 0.0, 126.0, AOT.max, AOT.min)                 # ys
    tt(out=S1, in0=Y0, in1=YS, op=AOT.is_equal)              # e0y
    tt(out=S2, in0=P0Y, in1=S1, op=AOT.mult)
    ts(S3, Y0, -1.0, None, AOT.is_equal)
    tt(out=S3, in0=P1Y, in1=S3, op=AOT.mult)
    tt(out=S2, in0=S2, in1=S3, op=AOT.add)                   # wy0 raw
    tt(out=S1, in0=P1Y, in1=S1, op=AOT.mult)                 # p1y*e0y
    ts(S3, Y0, 127.0, None, AOT.is_equal)
    tt(out=S3, in0=P0Y, in1=S3, op=AOT.mult)
    tt(out=S1, in0=S1, in1=S3, op=AOT.add)                   # wy1 raw
    tt(out=P0Y, in0=S2, in1=MSK, op=AOT.mult)                # wy0
    tt(out=P1Y, in0=S1, in1=MSK, op=AOT.mult)                # wy1
    WY0, WY1 = P0Y, P1Y

    # base flat index = ys*256 + xs
    BASE = S2
    ts(BASE, YS, 256.0, None, AOT.mult)
    tt(out=BASE, in0=BASE, in1=XS, op=AOT.add)

    # natural-layout per-corner planes, col = 4*j + z, z in {y0x0,y0x1,y1x0,y1x1}
    Wv = Mw[:, :].rearrange("p (j z) -> p j z", z=4)
    tt(out=Wv[:, :, 0], in0=WY0, in1=WSA, op=AOT.mult)
    tt(out=Wv[:, :, 1], in0=WY0, in1=WSB, op=AOT.mult)
    tt(out=Wv[:, :, 2], in0=WY1, in1=WSA, op=AOT.mult)
    tt(out=Wv[:, :, 3], in0=WY1, in1=WSB, op=AOT.mult)
    Iv = Mi[:, :].rearrange("p (j z) -> p j z", z=4)
    ts(Iv[:, :, 0], BASE, 0.0, None, AOT.add)
    ts(Iv[:, :, 1], BASE, 1.0, None, AOT.add)
    ts(Iv[:, :, 2], BASE, 256.0, None, AOT.add)
    ts(Iv[:, :, 3], BASE, 257.0, None, AOT.add)

    # ------------- transpose maps into gather layouts -------------
    # weights: WT[:, 128u + r] = w at partition delta = 4*(j-32u)+z
    for u in range(NU):
        psw = tpool.tile([128, 128], F32, tag="psw", name="psw")
        nc.tensor.transpose(psw[:, :], Mw[:, 128 * u:128 * (u + 1)],
                            cs["ID"][:, :])
        nc.vector.tensor_copy(out=WT[:, 128 * u:128 * (u + 1)], in_=psw[:, :])
        # indices: for each 16-col group g: psum[p16, r] -> strided int16
        # columns s = 8r + g of the wrapped layout (per-u base 1024u)
        for g in range(8):
            psi = tpool.tile([16, 128], F32, tag="psi", name="psi")
            c0 = 128 * u + 16 * g
            nc.tensor.transpose(psi[:, :], Mi[:, c0:c0 + 16], cs["ID"][:, :])
            dst = IX[0:16, 1024 * u:1024 * (u + 1)].rearrange(
                "p (r g) -> p r g", g=8)
            nc.vector.tensor_copy(out=dst[:, :, g], in_=psi[:, :])
        # replicate this u's wrapped indices to all 8 core groups right
        # away so u=0 gathers don't wait on the whole map prologue
        for k in range(1, 8):
            nc.sync.dma_start(
                out=IX[16 * k:16 * (k + 1), 1024 * u:1024 * (u + 1)],
                in_=IX[0:16, 1024 * u:1024 * (u + 1)])

    # ---------------- gather + blend ----------------
    if "gather" in skip:
        return
    # SWDGE caps one gather at ~1024 descriptors (2048 wedges the ring) ->
    # chunk = 1024 indices = 8 blocks (output rows) = one PSUM bank.
    for u in range(NU):
        # weights folded into the stationary one-hot: SW[p, 32*blk + m] =
        # S[p, m] * WT[p, 128u + blk] (free size 32 per block instead of
        # multiplying the 256-wide gathered rows)
        SW = gpool.tile([128, 4096], F16, tag="SW", name="SW")
        if "wmul" not in skip:
            sv = cs["S"][:, :].rearrange("p (a m) -> p a m", a=1)
            wv = WT[:, 128 * u:128 * (u + 1)].rearrange(
                "p (b o) -> p b o", o=1)
            b0, b1 = bass.broadcast_tensor_aps(sv, wv)
            nc.vector.tensor_tensor(
                out=SW[:, :].rearrange("p (b m) -> p b m", m=32),
                in0=b0, in1=b1, op=AOT.mult)
        else:
            nc.vector.memset(SW[:, 0:32], 0.0)
        for R in range(16):
            G = gpool.tile([128, 8, 256], F16, tag="G", name="G")
            scol = 1024 * u + 64 * R
            if "dg" in skip:
                nc.vector.memset(G[:, 0, 0:4], 0.0)  # keep tile alive
            else:
                nc.gpsimd.dma_gather(
                    out_ap=G[:, :, :],
                    in_ap=imgT[:, :],
                    idxs_ap=IX[:, scol:scol + 64],
                    num_idxs=1024,
                    num_idxs_reg=1024,
                    elem_size=CP,
                    queue_num=(16 * u + R) % 4,
                )
            if "blend" in skip:
                continue
            psb = bpool.tile([128, 512], F32, tag="psb", name="psb")
            for q in range(8):
                blk = 8 * R + q
                nc.tensor.matmul(
                    out=psb[32 * (q % 4):32 * (q % 4) + 32,
                            C * (q // 4):C * (q // 4) + C],
                    lhsT=SW[:, 32 * blk:32 * blk + 32],
                    rhs=G[:, q, 0:C],
                    start=True, stop=True,
                    tile_position=(0, 32 * (q % 4)))
            # drain PSUM through SBUF (DMA cannot read PSUM), then
            # 8 blocks -> 8 strided groups of 32 consecutive outT rows;
            # block q = 4*qc + qp sits at partitions 32*qp, col half qc,
            # and covers outT rows 1024*a2 + 256*qp + 32*u + [0, 32).
            ob = opool.tile([128, 2 * C], F32, tag="OB", name="OB")
            if R % 2 == 0:
                nc.scalar.copy(out=ob[:, :], in_=psb[:, 0:2 * C])
            else:
                nc.vector.tensor_copy(out=ob[:, :], in_=psb[:, 0:2 * C])
            for qc in range(2):
                a2 = 2 * R + qc
                dstv = outT[:, :].rearrange(
                    "(a qp u m) c -> a u qp m c", qp=4, u=8, m=32)
                nc.sync.dma_start(out=dstv[a2, u],
                                  in_=ob[:, C * qc:C * qc + C])


# ---------------- host side ----------------

_CONSTS = None
_PROGRAM = None


def _host_consts():
    global _CONSTS
    if _CONSTS is None:
        j = np.broadcast_to(np.arange(W, dtype=np.float32), (128, W)).copy()
        i = np.broadcast_to(np.arange(H, dtype=np.float32)[:, None],
                            (H, W)).copy()
        ident = np.eye(128, dtype=np.float32)
        s = np.zeros((128, 32), np.float16)
        s[np.arange(128), np.arange(128) // 4] = 1.0
        _CONSTS = {"cJ": j, "cI": i, "cID": ident, "cS": s}
    return _CONSTS


def make_in_maps(tensorInput, tensorFlow):
    consts = _host_consts()
    in_maps = []
    for b in range(B):
        imgT = np.zeros((HW, CP), np.float16)
        imgT[:, :C] = tensorInput[b].reshape(C, HW).T
        m = {"imgT": imgT,
             "flow": np.ascontiguousarray(tensorFlow[b])}
        m.update(consts)
        in_maps.append(m)
    return in_maps


def kernel(tensorInput, tensorFlow):
    from concourse.bass_utils import run_bass_kernel_spmd

    tensorInput = np.asarray(tensorInput, dtype=np.float32)
    tensorFlow = np.asarray(tensorFlow, dtype=np.float32)
    assert tensorInput.shape == (B, C, H, W)
    assert tensorFlow.shape == (B, 2, H, W)

    global _PROGRAM
    if _PROGRAM is None:
        _PROGRAM = build_program(n_iters=1)
    nc = _PROGRAM

    res = run_bass_kernel_spmd(nc, make_in_maps(tensorInput, tensorFlow),
                               list(range(N_CORES)))
    return np.stack([np.ascontiguousarray(
        res.results[b]["outT"].T).reshape(C, H, W) for b in range(B)], axis=0)
